# revision 1
# baseline (speedup 1.0000x reference)
"""Trainium2 Bass kernel for nn_Decoder (LSTM decoder + vocab projection).

Model (per reference):
  dec_emb = embed_W[outputs]                         # [L=64, B=128, H=256]
  step 0 uses GO embedding, steps 1..L-1 use dec_emb[1:]
  LSTM cell (PyTorch gate order i,f,g,o), 64 sequential steps
  logits = pred @ proj_W.T + proj_b                  # [64, 128, 32000]

Distribution over 8 NeuronCores:
  - LSTM replicated on every core (latency-bound; replication is free).
  - Projection tensor-parallel: vocab split 32000 -> 8 x 4000. Each core
    computes logits[:, :, c*4000:(c+1)*4000]; host concatenates + upcasts
    the bf16 device logits to f32.

v3 design — fully transposed LSTM state (feature-on-partition):
  - h, c live as [128 part = H-chunk, 2, B] tiles; h_new (bf16) IS the
    stationary operand for both the recurrent matmuls and the projection,
    so there are no PE transposes and nothing but the DVE tail on the
    h-recurrence critical path.
  - Gates computed transposed: 8 gate-chunks [128 gates, B] psum, each an
    accumulation group: K=1 bias row + 2 x-passes + 2 h-passes (all bf16,
    1 cycle/row).  Sigmoid reads psum per bank (tanh folded via
    shifted-sigmoid; h' = h/2 convention with 2x folded into host-scaled
    weights).
  - x rows gathered from a bf16 embed table via indirect DMA (batch on
    partition), then flipped to [H, B] with XBAR DMA transposes (112ns,
    on the DMA engines, off the critical path).
  - Projection: 8 chunks x 2 K-passes (bf16, N=500) into 4 psum banks;
    bias+downcast drain spread over DVE/Pool/ACT; bf16 logits streamed to
    DRAM (halves the dominant output-DMA stream).
"""

import numpy as np
import ml_dtypes

import concourse.bass as bass
import concourse.bacc as bacc
import concourse.mybir as mybir
import concourse.tile as tile
from concourse.bass import IndirectOffsetOnAxis
from concourse.bass_utils import run_bass_kernel_spmd

F32 = mybir.dt.float32
BF16 = mybir.dt.bfloat16
I32 = mybir.dt.int32

VOCAB = 32000
H = 256
L = 64
B = 128
G = 4 * H  # 1024 gates
GO_IDX = VOCAB - 1
NCORES = 8
VS = VOCAB // NCORES  # 4000 vocab columns per core
NP = 8  # projection N-chunks per step
PN = VS // NP  # 500 columns per projection matmul
NGC = 8  # gate chunks of 128

# proj-tail drain engine per chunk: D=DVE add, P=Pool add, A=ACT copy
# (ACT chunks get bias preloaded into psum via a K=1 bias-row matmul).
TAIL_ENG = ["D", "D", "A", "D", "D", "A", "D", "D"]


def emit_kernel(tc, io):
    nc = tc.nc
    from contextlib import ExitStack

    ctx = ExitStack()
    with ctx:
        const = ctx.enter_context(tc.tile_pool(name="const", bufs=1))
        xgp = ctx.enter_context(tc.tile_pool(name="xgp", bufs=12))
        xtp = ctx.enter_context(tc.tile_pool(name="xtp", bufs=4))
        state = ctx.enter_context(tc.tile_pool(name="state", bufs=2))
        work = ctx.enter_context(tc.tile_pool(name="work", bufs=2))
        lgp = ctx.enter_context(tc.tile_pool(name="lgp", bufs=4))
        g_psp = ctx.enter_context(tc.tile_pool(name="g_psp", bufs=1, space="PSUM"))
        pj_psp = ctx.enter_context(tc.tile_pool(name="pj_psp", bufs=6, space="PSUM"))

        # ---- constants into SBUF (small first) ----
        idx_sb = const.tile([B, L], I32)
        nc.sync.dma_start(out=idx_sb[:], in_=io["idx"][:])
        onesB_sb = const.tile([1, 128], BF16)
        nc.sync.dma_start(out=onesB_sb[:], in_=io["onesb"][:])
        bgate_sb = const.tile([1, G], BF16)
        nc.sync.dma_start(out=bgate_sb[:], in_=io["bgate"][:])
        pbrow_sb = const.tile([1, VS], BF16)
        nc.sync.dma_start(out=pbrow_sb[:], in_=io["pbrow"][:])
        h0t_sb = const.tile([128, 2, 128], BF16)
        nc.sync.dma_start(out=h0t_sb[:], in_=io["h0t"].rearrange("k p j -> p k j"))
        c0t_sb = const.tile([128, 2, 128], F32)
        nc.sync.dma_start(out=c0t_sb[:], in_=io["c0t"].rearrange("k p j -> p k j"))
        wc_sb = const.tile([128, 4 * G], BF16)  # [Whh.T k0, k1, Wih.T k0, k1]
        for j in range(4):
            nc.sync.dma_start(out=wc_sb[:, j * G : (j + 1) * G], in_=io["wc"][j])
        pbb_sb = const.tile([B, VS], F32)
        nc.scalar.dma_start(out=pbb_sb[:], in_=io["pbb"][:])
        pw_sb = const.tile([128, 2 * VS], BF16)  # proj_W.T chunks
        for j in range(2):
            nc.scalar.dma_start(out=pw_sb[:, j * VS : (j + 1) * VS], in_=io["pw"][j])

        embed = io["embed"]
        logits_out = io["logits"]

        # ---- embedding gathers (idx[:,0] = GO): out[p,:] = embed[idx[p,t],:]
        LOOKAHEAD = 12
        xg_tiles = [None] * L

        def gather(t):
            xg = xgp.tile([B, H], BF16, name=f"xg{t}", tag="xg")
            nc.gpsimd.indirect_dma_start(
                out=xg[:],
                out_offset=None,
                in_=embed[:],
                in_offset=IndirectOffsetOnAxis(ap=idx_sb[:, t : t + 1], axis=0),
            )
            xg_tiles[t] = xg

        for t0 in range(LOOKAHEAD):
            gather(t0)

        def emit_xt(t):
            """DMA-transpose gathered x rows [B, H] -> xT [H-chunk, 2, B].
            Issued on the SP queue ahead of the logits stream-out: deps
            (the gather, LOOKAHEAD steps out) are long satisfied, so these
            never block the queue."""
            xt = xtp.tile([128, 2, 128], BF16, name=f"xt{t}", tag="xt")
            for k in range(2):
                nc.sync.dma_start(
                    out=xt[:, k, :],
                    in_=xg_tiles[t][:, k * 128 : (k + 1) * 128],
                    transpose=True,
                )
            return xt

        def emit_xpart(xt):
            """Open gates psum group for the NEXT step: per gate-chunk, K=1
            bias row then the two x-passes. Off the h critical path."""
            g_ps = g_psp.tile([128, NGC, 128], F32, name="g_ps", tag="g")
            # accumulation flags are per 2KB psum bank (zero region): only
            # the bank's first matmul starts; later chunks auto-zero their
            # own range on first touch.
            for gc in range(NGC):
                nc.tensor.matmul(
                    g_ps[:, gc, :],
                    bgate_sb[:, gc * 128 : (gc + 1) * 128],
                    onesB_sb[:],
                    start=(gc % 4 == 0),
                    stop=False,
                )
                for j in (0, 1):  # W_ih.T K-chunks (wc slots 2,3)
                    nc.tensor.matmul(
                        g_ps[:, gc, :],
                        wc_sb[:, (2 + j) * G + gc * 128 : (2 + j) * G + (gc + 1) * 128],
                        xt[:, j, :],
                        start=False,
                        stop=False,
                    )
            return g_ps

        def emit_proj(hT_tile, lg, t):
            """All 8 chunks, each drained right after its matmuls so psum
            banks free early. Emitted AFTER the DVE tail, so the drain ops
            queue behind the recurrence on every engine."""
            for n in range(NP):
                pj = pj_psp.tile([128, 512], F32, name="pj", tag="pj")
                sl = slice(n * PN, (n + 1) * PN)
                if TAIL_ENG[n] == "A":
                    nc.tensor.matmul(
                        pj[:, :PN],
                        onesB_sb[:],
                        pbrow_sb[:, sl],
                        start=True,
                        stop=False,
                    )
                for k in range(2):
                    nc.tensor.matmul(
                        pj[:, :PN],
                        hT_tile[:, k, :],
                        pw_sb[:, k * VS + n * PN : k * VS + (n + 1) * PN],
                        start=(k == 0) and TAIL_ENG[n] != "A",
                        stop=(k == 1),
                    )
                if TAIL_ENG[n] == "A":
                    nc.scalar.copy(lg[:, sl], pj[:, :PN])
                elif TAIL_ENG[n] == "P":
                    nc.gpsimd.tensor_add(lg[:, sl], pj[:, :PN], pbb_sb[:, sl])
                else:
                    nc.vector.tensor_add(lg[:, sl], pj[:, :PN], pbb_sb[:, sl])
            nc.sync.dma_start(out=logits_out[t], in_=lg[:])

        # ---- prologue: x(0) is the GO row (idx[:,0]=GO_IDX) ----
        xt0 = emit_xt(0)
        g_cur = emit_xpart(xt0)
        hT = h0t_sb
        c_cur = c0t_sb
        prev = None

        MUL = mybir.AluOpType.mult
        ADD = mybir.AluOpType.add
        SUB = mybir.AluOpType.subtract
        AF = mybir.ActivationFunctionType

        for t in range(L):
            if t + LOOKAHEAD < L:
                gather(t + LOOKAHEAD)

            # (a) close gates(t): h-passes with hT = h(t-1) [H,B] bf16
            for gc in range(NGC):
                for j in (0, 1):  # W_hh.T K-chunks (wc slots 0,1)
                    nc.tensor.matmul(
                        g_cur[:, gc, :],
                        wc_sb[:, j * G + gc * 128 : j * G + (gc + 1) * 128],
                        hT[:, j, :],
                        start=False,
                        stop=(j == 1) and (gc % 4 == 3),
                    )

            # (b) xT(t+1) transposes (ready early; SP queue)
            if t + 1 < L:
                xt = emit_xt(t + 1)

            # (d) activations per bank half: sigmoid(i,f), tanh(g),
            # sigmoid(o) — all in the 'sigmoid_and_others' table set, so no
            # table reloads. Chunk order [i0 i1 f0 f1 | g0 g1 o0 o1].
            gact = work.tile([128, NGC, 128], F32, name="gact", tag="gact")
            nc.scalar.activation(gact[:, 0:4, :], g_cur[:, 0:4, :], AF.Sigmoid)
            nc.scalar.activation(gact[:, 4:6, :], g_cur[:, 4:6, :], AF.Tanh)
            nc.scalar.activation(gact[:, 6:8, :], g_cur[:, 6:8, :], AF.Sigmoid)

            # c = f*c + i*tg ; h = o*tanh(c) — plain TensorTensor ops on
            # Pool (the HW compiler rejects TensorScalarPtr there).
            fc = work.tile([128, 2, 128], F32, name="fc", tag="fc")
            itg = work.tile([128, 2, 128], F32, name="itg", tag="itg")
            c_new = state.tile([128, 2, 128], F32, name="c_new", tag="c")
            th = work.tile([128, 2, 128], F32, name="th", tag="th")
            h_new = state.tile([128, 2, 128], BF16, name="h_new", tag="h")
            nc.gpsimd.tensor_mul(fc[:], gact[:, 2:4, :], c_cur[:])
            nc.gpsimd.tensor_mul(itg[:], gact[:, 0:2, :], gact[:, 4:6, :])
            # c/th split per H-half pipelines Pool and ACT; h_new stays ONE
            # op so every next-step h-pass becomes ready simultaneously and
            # the scheduler keeps them bank-major (bank0 closes early -> the
            # sigmoid chain starts sooner). Pool touches SBUF only (PSUM is
            # off-limits to GPSIMD on real HW).
            nc.gpsimd.tensor_add(c_new[:], fc[:], itg[:])
            nc.scalar.activation(th[:], c_new[:], AF.Tanh)
            nc.gpsimd.tensor_mul(h_new[:], gact[:, 6:8, :], th[:])
            c_cur = c_new

            # (e) projection(t-1): mms + interleaved drains + stream-out
            if prev is not None:
                lg_prev = lgp.tile([B, VS], BF16, name="lg", tag="lg")
                emit_proj(prev, lg_prev, t - 1)

            # (f) open gates(t+1) LAST on PE: by now the sigmoids have read
            # gates(t), so one psum pair suffices (bufs=1) freeing banks
            # for the projection pipeline.
            g_next = emit_xpart(xt) if t + 1 < L else None
            prev = hT = h_new
            g_cur = g_next

        lg_last = lgp.tile([B, VS], BF16, name="lg", tag="lg")
        emit_proj(prev, lg_last, L - 1)


def build_program(reps=1):
    """Build + compile the Bacc program. reps>1 repeats the whole kernel
    body (for slope-based HW timing)."""
    nc = bacc.Bacc("TRN2", target_bir_lowering=False, debug=False,
                   enable_asserts=False)
    io = {
        "idx": nc.dram_tensor("idx", [B, L], I32, kind="ExternalInput")[:],
        "h0t": nc.dram_tensor("h0t", [2, 128, 128], BF16, kind="ExternalInput")[:],
        "c0t": nc.dram_tensor("c0t", [2, 128, 128], F32, kind="ExternalInput")[:],
        "wc": nc.dram_tensor("wc", [4, 128, G], BF16, kind="ExternalInput")[:],
        "bgate": nc.dram_tensor("bgate", [1, G], BF16, kind="ExternalInput")[:],
        "onesb": nc.dram_tensor("onesb", [1, 128], BF16, kind="ExternalInput")[:],
        "pw": nc.dram_tensor("pw", [2, 128, VS], BF16, kind="ExternalInput")[:],
        "pbb": nc.dram_tensor("pbb", [B, VS], F32, kind="ExternalInput")[:],
        "pbrow": nc.dram_tensor("pbrow", [1, VS], BF16, kind="ExternalInput")[:],
        "embed": nc.dram_tensor("embed", [VOCAB, H], BF16, kind="ExternalInput")[:],
        "logits": nc.dram_tensor("logits", [L, B, VS], BF16, kind="ExternalOutput")[:],
    }
    with tile.TileContext(nc) as tc:
        for _ in range(reps):
            emit_kernel(tc, io)
    nc.compile()
    return nc


def make_in_maps(inputs):
    bf = ml_dtypes.bfloat16
    outputs = np.asarray(inputs["outputs"])
    h0 = np.asarray(inputs["h0"], dtype=np.float32)
    c0 = np.asarray(inputs["c0"], dtype=np.float32)
    embed_W = np.asarray(inputs["embed_W"], dtype=np.float32)
    W_ih = np.asarray(inputs["W_ih"], dtype=np.float32)
    W_hh = np.asarray(inputs["W_hh"], dtype=np.float32)
    b = (np.asarray(inputs["b_ih"], dtype=np.float32)
         + np.asarray(inputs["b_hh"], dtype=np.float32))
    proj_W = np.asarray(inputs["proj_W"], dtype=np.float32)
    proj_b = np.asarray(inputs["proj_b"], dtype=np.float32)

    idx = outputs.T.astype(np.int64).copy()  # [B, L]
    idx[:, 0] = GO_IDX
    idx = np.clip(idx, 0, VOCAB - 1).astype(np.int32)

    WhhT = np.ascontiguousarray(W_hh.T)  # [256, 1024]
    WihT = np.ascontiguousarray(W_ih.T)
    wc = np.stack([WhhT[0:128], WhhT[128:256], WihT[0:128], WihT[128:256]])
    bgate = b.copy()
    wc = np.ascontiguousarray(wc).astype(bf)
    bgate = np.ascontiguousarray(bgate[None, :]).astype(bf)

    h0t = np.stack([h0.T[0:128], h0.T[128:256]]).astype(bf)  # [2,128,B]
    c0t = np.ascontiguousarray(np.stack([c0.T[0:128], c0.T[128:256]]))

    onesb = np.ones((1, 128), dtype=np.float32).astype(bf)
    pwT = np.ascontiguousarray(proj_W.T)  # [256, 32000]

    common = dict(idx=idx, h0t=h0t, c0t=c0t, wc=wc, bgate=bgate, onesb=onesb,
                  embed=np.ascontiguousarray(embed_W.astype(bf)))
    in_maps = []
    for c in range(NCORES):
        sl = slice(c * VS, (c + 1) * VS)
        in_maps.append(dict(
            common,
            pw=np.ascontiguousarray(
                np.stack([pwT[0:128, sl], pwT[128:256, sl]])).astype(bf),
            pbb=np.ascontiguousarray(np.tile(proj_b[None, sl], (B, 1))),
            pbrow=np.ascontiguousarray(proj_b[None, sl]).astype(bf),
        ))
    return in_maps


_NC_CACHE = {}


def postprocess(res) -> np.ndarray:
    return np.concatenate(
        [res.results[c]["logits"] for c in range(NCORES)], axis=2
    ).astype(np.float32)


def kernel(**inputs) -> np.ndarray:
    if "nc" not in _NC_CACHE:
        _NC_CACHE["nc"] = build_program()
    nc = _NC_CACHE["nc"]
    in_maps = make_in_maps(inputs)
    res = run_bass_kernel_spmd(nc, in_maps, list(range(NCORES)))
    return postprocess(res)



# revision 39
# speedup vs baseline: 122.2134x; 122.2134x over previous
"""Trainium2 Bass kernel for nn_Decoder (LSTM decoder + vocab projection).

Model (per reference):
  dec_emb = embed_W[outputs]                         # [L=64, B=128, H=256]
  step 0 uses GO embedding, steps 1..L-1 use dec_emb[1:]
  LSTM cell (PyTorch gate order i,f,g,o), 64 sequential steps
  logits = pred @ proj_W.T + proj_b                  # [64, 128, 32000]

Distribution over 8 NeuronCores:
  - LSTM replicated on every core (latency-bound; replication is free).
  - Projection tensor-parallel: vocab split 32000 -> 8 x 4000. Each core
    computes logits[:, :, c*4000:(c+1)*4000]; host concatenates + upcasts
    the bf16 device logits to f32.

v3 design — fully transposed LSTM state (feature-on-partition):
  - h, c live as [128 part = H-chunk, 2, B] tiles; h_new (bf16) IS the
    stationary operand for both the recurrent matmuls and the projection,
    so there are no PE transposes and nothing but the DVE tail on the
    h-recurrence critical path.
  - Gates computed transposed: 8 gate-chunks [128 gates, B] psum, each an
    accumulation group: K=1 bias row + 2 x-passes + 2 h-passes (all bf16,
    1 cycle/row).  Sigmoid reads psum per bank (tanh folded via
    shifted-sigmoid; h' = h/2 convention with 2x folded into host-scaled
    weights).
  - x rows gathered from a bf16 embed table via indirect DMA (batch on
    partition), then flipped to [H, B] with XBAR DMA transposes (112ns,
    on the DMA engines, off the critical path).
  - Projection: 8 chunks x 2 K-passes (bf16, N=500) into 4 psum banks;
    bias+downcast drain spread over DVE/Pool/ACT; bf16 logits streamed to
    DRAM (halves the dominant output-DMA stream).
"""

import numpy as np
import ml_dtypes

import concourse.bass as bass
import concourse.bacc as bacc
import concourse.mybir as mybir
import concourse.tile as tile
from concourse.bass import IndirectOffsetOnAxis
from concourse.bass_utils import run_bass_kernel_spmd

F32 = mybir.dt.float32
BF16 = mybir.dt.bfloat16
I32 = mybir.dt.int32

VOCAB = 32000
H = 256
L = 64
B = 128
G = 4 * H  # 1024 gates
GO_IDX = VOCAB - 1
NCORES = 8
VS = VOCAB // NCORES  # 4000 vocab columns per core
NP = 8  # projection N-chunks per step
PN = VS // NP  # 500 columns per projection matmul
NGC = 8  # gate chunks of 128

# proj-tail drain engine per chunk: D=DVE add, P=Pool add, A=ACT copy
# (ACT chunks get bias preloaded into psum via a K=1 bias-row matmul).
TAIL_ENG = ["D", "D", "A", "D", "D", "A", "D", "D"]


def emit_kernel(tc, io):
    nc = tc.nc
    from contextlib import ExitStack

    ctx = ExitStack()
    with ctx:
        const = ctx.enter_context(tc.tile_pool(name="const", bufs=1))
        xgp = ctx.enter_context(tc.tile_pool(name="xgp", bufs=12))
        xtp = ctx.enter_context(tc.tile_pool(name="xtp", bufs=4))
        state = ctx.enter_context(tc.tile_pool(name="state", bufs=2))
        work = ctx.enter_context(tc.tile_pool(name="work", bufs=2))
        lgp = ctx.enter_context(tc.tile_pool(name="lgp", bufs=4))
        g_psp = ctx.enter_context(tc.tile_pool(name="g_psp", bufs=1, space="PSUM"))
        pj_psp = ctx.enter_context(tc.tile_pool(name="pj_psp", bufs=6, space="PSUM"))

        # ---- constants into SBUF (small first) ----
        idx_sb = const.tile([B, L], I32)
        nc.sync.dma_start(out=idx_sb[:], in_=io["idx"][:])
        onesB_sb = const.tile([1, 128], BF16)
        nc.sync.dma_start(out=onesB_sb[:], in_=io["onesb"][:])
        bgate_sb = const.tile([1, G], BF16)
        nc.sync.dma_start(out=bgate_sb[:], in_=io["bgate"][:])
        pbrow_sb = const.tile([1, VS], BF16)
        nc.sync.dma_start(out=pbrow_sb[:], in_=io["pbrow"][:])
        h0t_sb = const.tile([128, 2, 128], BF16)
        nc.sync.dma_start(out=h0t_sb[:], in_=io["h0t"].rearrange("k p j -> p k j"))
        c0t_sb = const.tile([128, 2, 128], F32)
        nc.sync.dma_start(out=c0t_sb[:], in_=io["c0t"].rearrange("k p j -> p k j"))
        wc_sb = const.tile([128, 4 * G], BF16)  # [Whh.T k0, k1, Wih.T k0, k1]
        for j in range(4):
            nc.sync.dma_start(out=wc_sb[:, j * G : (j + 1) * G], in_=io["wc"][j])
        pbb_sb = const.tile([B, VS], F32)
        nc.scalar.dma_start(out=pbb_sb[:], in_=io["pbb"][:])
        pw_sb = const.tile([128, 2 * VS], BF16)  # proj_W.T chunks
        for j in range(2):
            nc.scalar.dma_start(out=pw_sb[:, j * VS : (j + 1) * VS], in_=io["pw"][j])

        embed = io["embed"]
        logits_out = io["logits"]

        # ---- embedding gathers (idx[:,0] = GO): out[p,:] = embed[idx[p,t],:]
        LOOKAHEAD = 12
        xg_tiles = [None] * L

        def gather(t):
            xg = xgp.tile([B, H], BF16, name=f"xg{t}", tag="xg")
            nc.gpsimd.indirect_dma_start(
                out=xg[:],
                out_offset=None,
                in_=embed[:],
                in_offset=IndirectOffsetOnAxis(ap=idx_sb[:, t : t + 1], axis=0),
            )
            xg_tiles[t] = xg

        for t0 in range(LOOKAHEAD):
            gather(t0)

        def emit_xt(t):
            """DMA-transpose gathered x rows [B, H] -> xT [H-chunk, 2, B].
            Issued on the SP queue ahead of the logits stream-out: deps
            (the gather, LOOKAHEAD steps out) are long satisfied, so these
            never block the queue."""
            xt = xtp.tile([128, 2, 128], BF16, name=f"xt{t}", tag="xt")
            for k in range(2):
                nc.sync.dma_start(
                    out=xt[:, k, :],
                    in_=xg_tiles[t][:, k * 128 : (k + 1) * 128],
                    transpose=True,
                )
            return xt

        def emit_xpart(xt):
            """Open gates psum group for the NEXT step: per gate-chunk, K=1
            bias row then the two x-passes. Off the h critical path."""
            g_ps = g_psp.tile([128, NGC, 128], F32, name="g_ps", tag="g")
            # accumulation flags are per 2KB psum bank (zero region): only
            # the bank's first matmul starts; later chunks auto-zero their
            # own range on first touch.
            for gc in range(NGC):
                nc.tensor.matmul(
                    g_ps[:, gc, :],
                    bgate_sb[:, gc * 128 : (gc + 1) * 128],
                    onesB_sb[:],
                    start=(gc % 4 == 0),
                    stop=False,
                )
                for j in (0, 1):  # W_ih.T K-chunks (wc slots 2,3)
                    nc.tensor.matmul(
                        g_ps[:, gc, :],
                        wc_sb[:, (2 + j) * G + gc * 128 : (2 + j) * G + (gc + 1) * 128],
                        xt[:, j, :],
                        start=False,
                        stop=False,
                    )
            return g_ps

        def emit_proj(hT_tile, lg, t):
            """All 8 chunks, each drained right after its matmuls so psum
            banks free early. Emitted AFTER the DVE tail, so the drain ops
            queue behind the recurrence on every engine."""
            for n in range(NP):
                pj = pj_psp.tile([128, 512], F32, name="pj", tag="pj")
                sl = slice(n * PN, (n + 1) * PN)
                if TAIL_ENG[n] == "A":
                    nc.tensor.matmul(
                        pj[:, :PN],
                        onesB_sb[:],
                        pbrow_sb[:, sl],
                        start=True,
                        stop=False,
                    )
                for k in range(2):
                    nc.tensor.matmul(
                        pj[:, :PN],
                        hT_tile[:, k, :],
                        pw_sb[:, k * VS + n * PN : k * VS + (n + 1) * PN],
                        start=(k == 0) and TAIL_ENG[n] != "A",
                        stop=(k == 1),
                    )
                if TAIL_ENG[n] == "A":
                    nc.scalar.copy(lg[:, sl], pj[:, :PN])
                elif TAIL_ENG[n] == "P":
                    nc.gpsimd.tensor_add(lg[:, sl], pj[:, :PN], pbb_sb[:, sl])
                else:
                    nc.vector.tensor_add(lg[:, sl], pj[:, :PN], pbb_sb[:, sl])
            nc.sync.dma_start(out=logits_out[t], in_=lg[:])

        # ---- prologue: x(0) is the GO row (idx[:,0]=GO_IDX) ----
        xt0 = emit_xt(0)
        g_cur = emit_xpart(xt0)
        hT = h0t_sb
        c_cur = c0t_sb
        prev = None

        MUL = mybir.AluOpType.mult
        ADD = mybir.AluOpType.add
        SUB = mybir.AluOpType.subtract
        AF = mybir.ActivationFunctionType

        for t in range(L):
            if t + LOOKAHEAD < L:
                gather(t + LOOKAHEAD)

            # (a) close gates(t): h-passes with hT = h(t-1) [H,B] bf16
            for gc in range(NGC):
                for j in (0, 1):  # W_hh.T K-chunks (wc slots 0,1)
                    nc.tensor.matmul(
                        g_cur[:, gc, :],
                        wc_sb[:, j * G + gc * 128 : j * G + (gc + 1) * 128],
                        hT[:, j, :],
                        start=False,
                        stop=(j == 1) and (gc % 4 == 3),
                    )

            # (b) xT(t+1) transposes (ready early; SP queue)
            if t + 1 < L:
                xt = emit_xt(t + 1)

            # (d) activations per bank half: sigmoid(i,f), tanh(g),
            # sigmoid(o) — all in the 'sigmoid_and_others' table set, so no
            # table reloads. Chunk order [i0 i1 f0 f1 | g0 g1 o0 o1].
            gact = work.tile([128, NGC, 128], F32, name="gact", tag="gact")
            nc.scalar.activation(gact[:, 0:4, :], g_cur[:, 0:4, :], AF.Sigmoid)
            nc.scalar.activation(gact[:, 4:6, :], g_cur[:, 4:6, :], AF.Tanh)
            nc.scalar.activation(gact[:, 6:8, :], g_cur[:, 6:8, :], AF.Sigmoid)

            # c = f*c + i*tg ; h = o*tanh(c) — plain TensorTensor ops on
            # Pool (the HW compiler rejects TensorScalarPtr there).
            fc = work.tile([128, 2, 128], F32, name="fc", tag="fc")
            itg = work.tile([128, 2, 128], F32, name="itg", tag="itg")
            c_new = state.tile([128, 2, 128], F32, name="c_new", tag="c")
            th = work.tile([128, 2, 128], F32, name="th", tag="th")
            h_new = state.tile([128, 2, 128], BF16, name="h_new", tag="h")
            nc.gpsimd.tensor_mul(fc[:], gact[:, 2:4, :], c_cur[:])
            nc.gpsimd.tensor_mul(itg[:], gact[:, 0:2, :], gact[:, 4:6, :])
            # c/th split per H-half pipelines Pool and ACT; h_new stays ONE
            # op so every next-step h-pass becomes ready simultaneously and
            # the scheduler keeps them bank-major (bank0 closes early -> the
            # sigmoid chain starts sooner). Pool touches SBUF only (PSUM is
            # off-limits to GPSIMD on real HW).
            nc.gpsimd.tensor_add(c_new[:], fc[:], itg[:])
            nc.scalar.activation(th[:], c_new[:], AF.Tanh)
            nc.gpsimd.tensor_mul(h_new[:], gact[:, 6:8, :], th[:])
            c_cur = c_new

            # (e) projection(t-1): mms + interleaved drains + stream-out
            if prev is not None:
                lg_prev = lgp.tile([B, VS], BF16, name="lg", tag="lg")
                emit_proj(prev, lg_prev, t - 1)

            # (f) open gates(t+1) LAST on PE: by now the sigmoids have read
            # gates(t), so one psum pair suffices (bufs=1) freeing banks
            # for the projection pipeline.
            g_next = emit_xpart(xt) if t + 1 < L else None
            prev = hT = h_new
            g_cur = g_next

        lg_last = lgp.tile([B, VS], BF16, name="lg", tag="lg")
        emit_proj(prev, lg_last, L - 1)


def build_program(reps=1):
    """Build + compile the Bacc program. reps>1 repeats the whole kernel
    body (for slope-based HW timing)."""
    nc = bacc.Bacc("TRN2", target_bir_lowering=False, debug=False,
                   enable_asserts=False)
    io = {
        "idx": nc.dram_tensor("idx", [B, L], I32, kind="ExternalInput")[:],
        "h0t": nc.dram_tensor("h0t", [2, 128, 128], BF16, kind="ExternalInput")[:],
        "c0t": nc.dram_tensor("c0t", [2, 128, 128], F32, kind="ExternalInput")[:],
        "wc": nc.dram_tensor("wc", [4, 128, G], BF16, kind="ExternalInput")[:],
        "bgate": nc.dram_tensor("bgate", [1, G], BF16, kind="ExternalInput")[:],
        "onesb": nc.dram_tensor("onesb", [1, 128], BF16, kind="ExternalInput")[:],
        "pw": nc.dram_tensor("pw", [2, 128, VS], BF16, kind="ExternalInput")[:],
        "pbb": nc.dram_tensor("pbb", [B, VS], F32, kind="ExternalInput")[:],
        "pbrow": nc.dram_tensor("pbrow", [1, VS], BF16, kind="ExternalInput")[:],
        "embed": nc.dram_tensor("embed", [VOCAB, H], BF16, kind="ExternalInput")[:],
        "logits": nc.dram_tensor("logits", [L, B, VS], BF16, kind="ExternalOutput")[:],
    }
    with tile.TileContext(nc) as tc:
        for _ in range(reps):
            emit_kernel(tc, io)
    nc.compile()
    return nc


def make_in_maps(inputs):
    bf = ml_dtypes.bfloat16
    outputs = np.asarray(inputs["outputs"])
    h0 = np.asarray(inputs["h0"], dtype=np.float32)
    c0 = np.asarray(inputs["c0"], dtype=np.float32)
    embed_W = np.asarray(inputs["embed_W"], dtype=np.float32)
    W_ih = np.asarray(inputs["W_ih"], dtype=np.float32)
    W_hh = np.asarray(inputs["W_hh"], dtype=np.float32)
    b = (np.asarray(inputs["b_ih"], dtype=np.float32)
         + np.asarray(inputs["b_hh"], dtype=np.float32))
    proj_W = np.asarray(inputs["proj_W"], dtype=np.float32)
    proj_b = np.asarray(inputs["proj_b"], dtype=np.float32)

    idx = outputs.T.astype(np.int64).copy()  # [B, L]
    idx[:, 0] = GO_IDX
    idx = np.clip(idx, 0, VOCAB - 1).astype(np.int32)

    WhhT = np.ascontiguousarray(W_hh.T)  # [256, 1024]
    WihT = np.ascontiguousarray(W_ih.T)
    wc = np.stack([WhhT[0:128], WhhT[128:256], WihT[0:128], WihT[128:256]])
    bgate = b.copy()
    wc = np.ascontiguousarray(wc).astype(bf)
    bgate = np.ascontiguousarray(bgate[None, :]).astype(bf)

    h0t = np.stack([h0.T[0:128], h0.T[128:256]]).astype(bf)  # [2,128,B]
    c0t = np.ascontiguousarray(np.stack([c0.T[0:128], c0.T[128:256]]))

    onesb = np.ones((1, 128), dtype=np.float32).astype(bf)
    pwT = np.ascontiguousarray(proj_W.T)  # [256, 32000]

    common = dict(idx=idx, h0t=h0t, c0t=c0t, wc=wc, bgate=bgate, onesb=onesb,
                  embed=np.ascontiguousarray(embed_W.astype(bf)))
    in_maps = []
    for c in range(NCORES):
        sl = slice(c * VS, (c + 1) * VS)
        in_maps.append(dict(
            common,
            pw=np.ascontiguousarray(
                np.stack([pwT[0:128, sl], pwT[128:256, sl]])).astype(bf),
            pbb=np.ascontiguousarray(np.tile(proj_b[None, sl], (B, 1))),
            pbrow=np.ascontiguousarray(proj_b[None, sl]).astype(bf),
        ))
    return in_maps


_NC_CACHE = {}


def postprocess(res) -> np.ndarray:
    return np.concatenate(
        [res.results[c]["logits"] for c in range(NCORES)], axis=2
    ).astype(np.float32)


def kernel(**inputs) -> np.ndarray:
    if "nc" not in _NC_CACHE:
        _NC_CACHE["nc"] = build_program()
    nc = _NC_CACHE["nc"]
    in_maps = make_in_maps(inputs)
    res = run_bass_kernel_spmd(nc, in_maps, list(range(NCORES)))
    return postprocess(res)



# revision 51
# speedup vs baseline: 122.3700x; 1.0013x over previous
"""Trainium2 Bass kernel for nn_Decoder (LSTM decoder + vocab projection).

Model (per reference):
  dec_emb = embed_W[outputs]                         # [L=64, B=128, H=256]
  step 0 uses GO embedding, steps 1..L-1 use dec_emb[1:]
  LSTM cell (PyTorch gate order i,f,g,o), 64 sequential steps
  logits = pred @ proj_W.T + proj_b                  # [64, 128, 32000]

Distribution over 8 NeuronCores:
  - LSTM replicated on every core (latency-bound; replication is free).
  - Projection tensor-parallel: vocab split 32000 -> 8 x 4000. Each core
    computes logits[:, :, c*4000:(c+1)*4000]; host concatenates + upcasts
    the bf16 device logits to f32.

v3 design — fully transposed LSTM state (feature-on-partition):
  - h, c live as [128 part = H-chunk, 2, B] tiles; h_new (bf16) IS the
    stationary operand for both the recurrent matmuls and the projection,
    so there are no PE transposes and nothing but the DVE tail on the
    h-recurrence critical path.
  - Gates computed transposed: 8 gate-chunks [128 gates, B] psum, each an
    accumulation group: K=1 bias row + 2 x-passes + 2 h-passes (all bf16,
    1 cycle/row).  Sigmoid reads psum per bank (tanh folded via
    shifted-sigmoid; h' = h/2 convention with 2x folded into host-scaled
    weights).
  - x rows gathered from a bf16 embed table via indirect DMA (batch on
    partition), then flipped to [H, B] with XBAR DMA transposes (112ns,
    on the DMA engines, off the critical path).
  - Projection: 8 chunks x 2 K-passes (bf16, N=500) into 4 psum banks;
    bias+downcast drain spread over DVE/Pool/ACT; bf16 logits streamed to
    DRAM (halves the dominant output-DMA stream).
"""

import numpy as np
import ml_dtypes

import concourse.bass as bass
import concourse.bacc as bacc
import concourse.mybir as mybir
import concourse.tile as tile
from concourse.bass import IndirectOffsetOnAxis
from concourse.bass_utils import run_bass_kernel_spmd

F32 = mybir.dt.float32
BF16 = mybir.dt.bfloat16
I32 = mybir.dt.int32

VOCAB = 32000
H = 256
L = 64
B = 128
G = 4 * H  # 1024 gates
GO_IDX = VOCAB - 1
NCORES = 8
VS = VOCAB // NCORES  # 4000 vocab columns per core
NP = 8  # projection N-chunks per step
PN = VS // NP  # 500 columns per projection matmul
NGC = 8  # gate chunks of 128

# proj-tail drain engine per chunk: D=DVE add, P=Pool add, A=ACT copy
# (ACT chunks get bias preloaded into psum via a K=1 bias-row matmul).
TAIL_ENG = ["D", "D", "A", "D", "D", "A", "D", "D"]


def emit_kernel(tc, io):
    nc = tc.nc
    from contextlib import ExitStack

    ctx = ExitStack()
    with ctx:
        const = ctx.enter_context(tc.tile_pool(name="const", bufs=1))
        xgp = ctx.enter_context(tc.tile_pool(name="xgp", bufs=12))
        xtp = ctx.enter_context(tc.tile_pool(name="xtp", bufs=4))
        state = ctx.enter_context(tc.tile_pool(name="state", bufs=2))
        work = ctx.enter_context(tc.tile_pool(name="work", bufs=2))
        lgp = ctx.enter_context(tc.tile_pool(name="lgp", bufs=4))
        g_psp = ctx.enter_context(tc.tile_pool(name="g_psp", bufs=1, space="PSUM"))
        pj_psp = ctx.enter_context(tc.tile_pool(name="pj_psp", bufs=6, space="PSUM"))

        # ---- constants into SBUF (small first) ----
        idx_sb = const.tile([B, L], I32)
        nc.sync.dma_start(out=idx_sb[:], in_=io["idx"][:])
        onesB_sb = const.tile([1, 128], BF16)
        nc.sync.dma_start(out=onesB_sb[:], in_=io["onesb"][:])
        bgate_sb = const.tile([1, G], BF16)
        nc.sync.dma_start(out=bgate_sb[:], in_=io["bgate"][:])
        pbrow_sb = const.tile([1, VS], BF16)
        nc.sync.dma_start(out=pbrow_sb[:], in_=io["pbrow"][:])
        h0t_sb = const.tile([128, 2, 128], BF16)
        nc.sync.dma_start(out=h0t_sb[:], in_=io["h0t"].rearrange("k p j -> p k j"))
        c0t_sb = const.tile([128, 2, 128], F32)
        nc.sync.dma_start(out=c0t_sb[:], in_=io["c0t"].rearrange("k p j -> p k j"))
        wc_sb = const.tile([128, 4 * G], BF16)  # [Whh.T k0, k1, Wih.T k0, k1]
        for j in range(4):
            nc.sync.dma_start(out=wc_sb[:, j * G : (j + 1) * G], in_=io["wc"][j])
        pbb_sb = const.tile([B, VS], F32)
        nc.scalar.dma_start(out=pbb_sb[:], in_=io["pbb"][:])
        pw_sb = const.tile([128, 2 * VS], BF16)  # proj_W.T chunks
        for j in range(2):
            nc.scalar.dma_start(out=pw_sb[:, j * VS : (j + 1) * VS], in_=io["pw"][j])

        embed = io["embed"]
        logits_out = io["logits"]

        # ---- embedding gathers (idx[:,0] = GO): out[p,:] = embed[idx[p,t],:]
        GB = 1  # tokens per gather batch
        LOOKAHEAD = 12
        xg_tiles = [None] * L

        def gather(t):
            xg = xgp.tile([B, H], BF16, name=f"xg{t}", tag="xg")
            nc.gpsimd.indirect_dma_start(
                out=xg[:],
                out_offset=None,
                in_=embed[:],
                in_offset=IndirectOffsetOnAxis(ap=idx_sb[:, t : t + 1], axis=0),
            )
            xg_tiles[t] = xg

        for t0 in range(LOOKAHEAD):
            gather(t0)

        def emit_xt(t):
            """DMA-transpose gathered x rows [B, H] -> xT [H-chunk, 2, B].
            Issued on the SP queue ahead of the logits stream-out: deps
            (the gather, LOOKAHEAD steps out) are long satisfied, so these
            never block the queue."""
            xt = xtp.tile([128, 2, 128], BF16, name=f"xt{t}", tag="xt")
            for k in range(2):
                nc.sync.dma_start(
                    out=xt[:, k, :],
                    in_=xg_tiles[t][:, k * 128 : (k + 1) * 128],
                    transpose=True,
                )
            return xt

        def emit_xpart(xt):
            """Open gates psum group for the NEXT step: per gate-chunk, K=1
            bias row then the two x-passes. Off the h critical path."""
            g_ps = g_psp.tile([128, NGC, 128], F32, name="g_ps", tag="g")
            # accumulation flags are per 2KB psum bank (zero region): only
            # the bank's first matmul starts; later chunks auto-zero their
            # own range on first touch.
            for gc in range(NGC):
                nc.tensor.matmul(
                    g_ps[:, gc, :],
                    bgate_sb[:, gc * 128 : (gc + 1) * 128],
                    onesB_sb[:],
                    start=(gc % 4 == 0),
                    stop=False,
                )
                for j in (0, 1):  # W_ih.T K-chunks (wc slots 2,3)
                    nc.tensor.matmul(
                        g_ps[:, gc, :],
                        wc_sb[:, (2 + j) * G + gc * 128 : (2 + j) * G + (gc + 1) * 128],
                        xt[:, j, :],
                        start=False,
                        stop=False,
                    )
            return g_ps

        def emit_proj(hT_tile, lg, t):
            """All 8 chunks, each drained right after its matmuls so psum
            banks free early. Emitted AFTER the DVE tail, so the drain ops
            queue behind the recurrence on every engine."""
            for n in range(NP):
                pj = pj_psp.tile([128, 512], F32, name="pj", tag="pj")
                sl = slice(n * PN, (n + 1) * PN)
                if TAIL_ENG[n] == "A":
                    nc.tensor.matmul(
                        pj[:, :PN],
                        onesB_sb[:],
                        pbrow_sb[:, sl],
                        start=True,
                        stop=False,
                    )
                for k in range(2):
                    nc.tensor.matmul(
                        pj[:, :PN],
                        hT_tile[:, k, :],
                        pw_sb[:, k * VS + n * PN : k * VS + (n + 1) * PN],
                        start=(k == 0) and TAIL_ENG[n] != "A",
                        stop=(k == 1),
                    )
                if TAIL_ENG[n] == "A":
                    nc.scalar.copy(lg[:, sl], pj[:, :PN])
                elif TAIL_ENG[n] == "P":
                    nc.gpsimd.tensor_add(lg[:, sl], pj[:, :PN], pbb_sb[:, sl])
                else:
                    nc.vector.tensor_add(lg[:, sl], pj[:, :PN], pbb_sb[:, sl])
            nc.sync.dma_start(out=logits_out[t], in_=lg[:])

        # ---- prologue: x(0) is the GO row (idx[:,0]=GO_IDX) ----
        xt0 = emit_xt(0)
        g_cur = emit_xpart(xt0)
        hT = h0t_sb
        c_cur = c0t_sb
        prev = None

        MUL = mybir.AluOpType.mult
        ADD = mybir.AluOpType.add
        SUB = mybir.AluOpType.subtract
        AF = mybir.ActivationFunctionType

        for t in range(L):
            # (a) close gates(t): h-passes with hT = h(t-1) [H,B] bf16
            for gc in range(NGC):
                for j in (0, 1):  # W_hh.T K-chunks (wc slots 0,1)
                    nc.tensor.matmul(
                        g_cur[:, gc, :],
                        wc_sb[:, j * G + gc * 128 : j * G + (gc + 1) * 128],
                        hT[:, j, :],
                        start=False,
                        stop=(j == 1) and (gc % 4 == 3),
                    )

            # (b) xT(t+1) transposes (ready early; SP queue)
            if t + 1 < L:
                xt = emit_xt(t + 1)

            # (d) activations per bank half: sigmoid(i,f), tanh(g),
            # sigmoid(o) — all in the 'sigmoid_and_others' table set, so no
            # table reloads. Chunk order [i0 i1 f0 f1 | g0 g1 o0 o1].
            gact = work.tile([128, NGC, 128], F32, name="gact", tag="gact")
            nc.scalar.activation(gact[:, 0:4, :], g_cur[:, 0:4, :], AF.Sigmoid)
            nc.scalar.activation(gact[:, 4:6, :], g_cur[:, 4:6, :], AF.Tanh)
            nc.scalar.activation(gact[:, 6:8, :], g_cur[:, 6:8, :], AF.Sigmoid)

            # c = f*c + i*tg ; h = o*tanh(c) — plain TensorTensor ops on
            # Pool (the HW compiler rejects TensorScalarPtr there).
            fc = work.tile([128, 2, 128], F32, name="fc", tag="fc")
            itg = work.tile([128, 2, 128], F32, name="itg", tag="itg")
            c_new = state.tile([128, 2, 128], F32, name="c_new", tag="c")
            th = work.tile([128, 2, 128], F32, name="th", tag="th")
            h_new = state.tile([128, 2, 128], BF16, name="h_new", tag="h")
            nc.gpsimd.tensor_mul(fc[:], gact[:, 2:4, :], c_cur[:])
            # itg on DVE: runs concurrently with fc (Pool), shortening the
            # serial cell chain; it precedes the proj drains in the DVE
            # stream but its input (tanh g) is ready early enough that the
            # drains are not materially delayed.
            nc.vector.tensor_mul(itg[:], gact[:, 0:2, :], gact[:, 4:6, :])
            # c/th split per H-half pipelines Pool and ACT; h_new stays ONE
            # op so every next-step h-pass becomes ready simultaneously and
            # the scheduler keeps them bank-major (bank0 closes early -> the
            # sigmoid chain starts sooner). Pool touches SBUF only (PSUM is
            # off-limits to GPSIMD on real HW).
            nc.gpsimd.tensor_add(c_new[:], fc[:], itg[:])
            nc.scalar.activation(th[:], c_new[:], AF.Tanh)
            nc.gpsimd.tensor_mul(h_new[:], gact[:, 6:8, :], th[:])
            c_cur = c_new

            # (e) projection(t-1): mms + interleaved drains + stream-out
            if prev is not None:
                lg_prev = lgp.tile([B, VS], BF16, name="lg", tag="lg")
                emit_proj(prev, lg_prev, t - 1)

            # (f) open gates(t+1) LAST on PE: by now the sigmoids have read
            # gates(t), so one psum pair suffices (bufs=1) freeing banks
            # for the projection pipeline.
            g_next = emit_xpart(xt) if t + 1 < L else None

            # gather LAST: its SWDGE generation (~1us on the Pool engine)
            # sits AFTER the cell ops in the Pool stream.
            if t + LOOKAHEAD < L:
                gather(t + LOOKAHEAD)

            prev = hT = h_new
            g_cur = g_next

        lg_last = lgp.tile([B, VS], BF16, name="lg", tag="lg")
        emit_proj(prev, lg_last, L - 1)


def build_program(reps=1):
    """Build + compile the Bacc program. reps>1 repeats the whole kernel
    body (for slope-based HW timing)."""
    nc = bacc.Bacc("TRN2", target_bir_lowering=False, debug=False,
                   enable_asserts=False)
    io = {
        "idx": nc.dram_tensor("idx", [B, L], I32, kind="ExternalInput")[:],
        "h0t": nc.dram_tensor("h0t", [2, 128, 128], BF16, kind="ExternalInput")[:],
        "c0t": nc.dram_tensor("c0t", [2, 128, 128], F32, kind="ExternalInput")[:],
        "wc": nc.dram_tensor("wc", [4, 128, G], BF16, kind="ExternalInput")[:],
        "bgate": nc.dram_tensor("bgate", [1, G], BF16, kind="ExternalInput")[:],
        "onesb": nc.dram_tensor("onesb", [1, 128], BF16, kind="ExternalInput")[:],
        "pw": nc.dram_tensor("pw", [2, 128, VS], BF16, kind="ExternalInput")[:],
        "pbb": nc.dram_tensor("pbb", [B, VS], F32, kind="ExternalInput")[:],
        "pbrow": nc.dram_tensor("pbrow", [1, VS], BF16, kind="ExternalInput")[:],
        "embed": nc.dram_tensor("embed", [VOCAB, H], BF16, kind="ExternalInput")[:],
        "logits": nc.dram_tensor("logits", [L, B, VS], BF16, kind="ExternalOutput")[:],
    }
    with tile.TileContext(nc) as tc:
        for _ in range(reps):
            emit_kernel(tc, io)
    nc.compile()
    return nc


def make_in_maps(inputs):
    bf = ml_dtypes.bfloat16
    outputs = np.asarray(inputs["outputs"])
    h0 = np.asarray(inputs["h0"], dtype=np.float32)
    c0 = np.asarray(inputs["c0"], dtype=np.float32)
    embed_W = np.asarray(inputs["embed_W"], dtype=np.float32)
    W_ih = np.asarray(inputs["W_ih"], dtype=np.float32)
    W_hh = np.asarray(inputs["W_hh"], dtype=np.float32)
    b = (np.asarray(inputs["b_ih"], dtype=np.float32)
         + np.asarray(inputs["b_hh"], dtype=np.float32))
    proj_W = np.asarray(inputs["proj_W"], dtype=np.float32)
    proj_b = np.asarray(inputs["proj_b"], dtype=np.float32)

    idx = outputs.T.astype(np.int64).copy()  # [B, L]
    idx[:, 0] = GO_IDX
    idx = np.clip(idx, 0, VOCAB - 1).astype(np.int32)

    WhhT = np.ascontiguousarray(W_hh.T)  # [256, 1024]
    WihT = np.ascontiguousarray(W_ih.T)
    wc = np.stack([WhhT[0:128], WhhT[128:256], WihT[0:128], WihT[128:256]])
    bgate = b.copy()
    wc = np.ascontiguousarray(wc).astype(bf)
    bgate = np.ascontiguousarray(bgate[None, :]).astype(bf)

    h0t = np.stack([h0.T[0:128], h0.T[128:256]]).astype(bf)  # [2,128,B]
    c0t = np.ascontiguousarray(np.stack([c0.T[0:128], c0.T[128:256]]))

    onesb = np.ones((1, 128), dtype=np.float32).astype(bf)
    pwT = np.ascontiguousarray(proj_W.T)  # [256, 32000]

    common = dict(idx=idx, h0t=h0t, c0t=c0t, wc=wc, bgate=bgate, onesb=onesb,
                  embed=np.ascontiguousarray(embed_W.astype(bf)))
    in_maps = []
    for c in range(NCORES):
        sl = slice(c * VS, (c + 1) * VS)
        in_maps.append(dict(
            common,
            pw=np.ascontiguousarray(
                np.stack([pwT[0:128, sl], pwT[128:256, sl]])).astype(bf),
            pbb=np.ascontiguousarray(np.tile(proj_b[None, sl], (B, 1))),
            pbrow=np.ascontiguousarray(proj_b[None, sl]).astype(bf),
        ))
    return in_maps


_NC_CACHE = {}


def postprocess(res) -> np.ndarray:
    return np.concatenate(
        [res.results[c]["logits"] for c in range(NCORES)], axis=2
    ).astype(np.float32)


def kernel(**inputs) -> np.ndarray:
    if "nc" not in _NC_CACHE:
        _NC_CACHE["nc"] = build_program()
    nc = _NC_CACHE["nc"]
    in_maps = make_in_maps(inputs)
    res = run_bass_kernel_spmd(nc, in_maps, list(range(NCORES)))
    return postprocess(res)



# revision 67
# speedup vs baseline: 123.0162x; 1.0053x over previous
"""Trainium2 Bass kernel for nn_Decoder (LSTM decoder + vocab projection).

Model (per reference):
  dec_emb = embed_W[outputs]                         # [L=64, B=128, H=256]
  step 0 uses GO embedding, steps 1..L-1 use dec_emb[1:]
  LSTM cell (PyTorch gate order i,f,g,o), 64 sequential steps
  logits = pred @ proj_W.T + proj_b                  # [64, 128, 32000]

Distribution over 8 NeuronCores:
  - LSTM replicated on every core (latency-bound; replication is free).
  - Projection tensor-parallel: vocab split 32000 -> 8 x 4000. Each core
    computes logits[:, :, c*4000:(c+1)*4000]; host concatenates + upcasts
    the bf16 device logits to f32.

v3 design — fully transposed LSTM state (feature-on-partition):
  - h, c live as [128 part = H-chunk, 2, B] tiles; h_new (bf16) IS the
    stationary operand for both the recurrent matmuls and the projection,
    so there are no PE transposes and nothing but the DVE tail on the
    h-recurrence critical path.
  - Gates computed transposed: 8 gate-chunks [128 gates, B] psum, each an
    accumulation group: K=1 bias row + 2 x-passes + 2 h-passes (all bf16,
    1 cycle/row).  Sigmoid reads psum per bank (tanh folded via
    shifted-sigmoid; h' = h/2 convention with 2x folded into host-scaled
    weights).
  - x rows gathered from a bf16 embed table via indirect DMA (batch on
    partition), then flipped to [H, B] with XBAR DMA transposes (112ns,
    on the DMA engines, off the critical path).
  - Projection: 8 chunks x 2 K-passes (bf16, N=500) into 4 psum banks;
    bias+downcast drain spread over DVE/Pool/ACT; bf16 logits streamed to
    DRAM (halves the dominant output-DMA stream).
"""

import numpy as np
import ml_dtypes

import concourse.bass as bass
import concourse.bacc as bacc
import concourse.mybir as mybir
import concourse.tile as tile
from concourse.bass import IndirectOffsetOnAxis
from concourse.bass_utils import run_bass_kernel_spmd

F32 = mybir.dt.float32
BF16 = mybir.dt.bfloat16
I32 = mybir.dt.int32

VOCAB = 32000
H = 256
L = 64
B = 128
G = 4 * H  # 1024 gates
GO_IDX = VOCAB - 1
NCORES = 8
VS = VOCAB // NCORES  # 4000 vocab columns per core
NP = 8  # projection N-chunks per step
PN = VS // NP  # 500 columns per projection matmul
NGC = 8  # gate chunks of 128

# proj-tail drain engine per chunk: D=DVE add, P=Pool add, A=ACT copy
# (ACT chunks get bias preloaded into psum via a K=1 bias-row matmul).
TAIL_ENG = ["D", "D", "A", "D", "D", "A", "D", "D"]


def emit_kernel(tc, io):
    nc = tc.nc
    from contextlib import ExitStack

    ctx = ExitStack()
    with ctx:
        const = ctx.enter_context(tc.tile_pool(name="const", bufs=1))
        xgp = ctx.enter_context(tc.tile_pool(name="xgp", bufs=12))
        xtp = ctx.enter_context(tc.tile_pool(name="xtp", bufs=4))
        state = ctx.enter_context(tc.tile_pool(name="state", bufs=2))
        work = ctx.enter_context(tc.tile_pool(name="work", bufs=2))
        lgp = ctx.enter_context(tc.tile_pool(name="lgp", bufs=4))
        g_psp = ctx.enter_context(tc.tile_pool(name="g_psp", bufs=1, space="PSUM"))
        pj_psp = ctx.enter_context(tc.tile_pool(name="pj_psp", bufs=6, space="PSUM"))

        # ---- constants into SBUF (small first) ----
        idx_sb = const.tile([B, L], I32)
        nc.sync.dma_start(out=idx_sb[:], in_=io["idx"][:])
        onesB_sb = const.tile([1, 128], BF16)
        nc.sync.dma_start(out=onesB_sb[:], in_=io["onesb"][:])
        bgate_sb = const.tile([1, G], BF16)
        nc.sync.dma_start(out=bgate_sb[:], in_=io["bgate"][:])
        pbrow_sb = const.tile([1, VS], BF16)
        nc.sync.dma_start(out=pbrow_sb[:], in_=io["pbrow"][:])
        h0t_sb = const.tile([128, 2, 128], BF16)
        nc.sync.dma_start(out=h0t_sb[:], in_=io["h0t"].rearrange("k p j -> p k j"))
        c0t_sb = const.tile([128, 2, 128], F32)
        nc.sync.dma_start(out=c0t_sb[:], in_=io["c0t"].rearrange("k p j -> p k j"))
        wc_sb = const.tile([128, 4 * G], BF16)  # [Whh.T k0, k1, Wih.T k0, k1]
        for j in range(4):
            nc.sync.dma_start(out=wc_sb[:, j * G : (j + 1) * G], in_=io["wc"][j])
        pbb_sb = const.tile([B, VS], F32)
        nc.scalar.dma_start(out=pbb_sb[:], in_=io["pbb"][:])
        pw_sb = const.tile([128, 2 * VS], BF16)  # proj_W.T chunks
        for j in range(2):
            nc.scalar.dma_start(out=pw_sb[:, j * VS : (j + 1) * VS], in_=io["pw"][j])

        embed = io["embed"]
        logits_out = io["logits"]

        # ---- embedding gathers (idx[:,0] = GO): out[p,:] = embed[idx[p,t],:]
        GB = 1  # tokens per gather batch
        LOOKAHEAD = 12
        xg_tiles = [None] * L

        def gather(t):
            xg = xgp.tile([B, H], BF16, name=f"xg{t}", tag="xg")
            nc.gpsimd.indirect_dma_start(
                out=xg[:],
                out_offset=None,
                in_=embed[:],
                in_offset=IndirectOffsetOnAxis(ap=idx_sb[:, t : t + 1], axis=0),
            )
            xg_tiles[t] = xg

        for t0 in range(LOOKAHEAD):
            gather(t0)

        def emit_xt(t):
            """DMA-transpose gathered x rows [B, H] -> xT [H-chunk, 2, B].
            Issued on the SP queue ahead of the logits stream-out: deps
            (the gather, LOOKAHEAD steps out) are long satisfied, so these
            never block the queue."""
            xt = xtp.tile([128, 2, 128], BF16, name=f"xt{t}", tag="xt")
            for k in range(2):
                nc.sync.dma_start(
                    out=xt[:, k, :],
                    in_=xg_tiles[t][:, k * 128 : (k + 1) * 128],
                    transpose=True,
                )
            return xt

        def emit_xpart(xt):
            """Open gates psum group for the NEXT step: per gate-chunk, K=1
            bias row then the two x-passes. Off the h critical path."""
            g_ps = g_psp.tile([128, NGC, 128], F32, name="g_ps", tag="g")
            # accumulation flags are per 2KB psum bank (zero region): only
            # the bank's first matmul starts; later chunks auto-zero their
            # own range on first touch.
            for gc in range(NGC):
                nc.tensor.matmul(
                    g_ps[:, gc, :],
                    bgate_sb[:, gc * 128 : (gc + 1) * 128],
                    onesB_sb[:],
                    start=(gc % 4 == 0),
                    stop=False,
                )
                for j in (0, 1):  # W_ih.T K-chunks (wc slots 2,3)
                    nc.tensor.matmul(
                        g_ps[:, gc, :],
                        wc_sb[:, (2 + j) * G + gc * 128 : (2 + j) * G + (gc + 1) * 128],
                        xt[:, j, :],
                        start=False,
                        stop=False,
                    )
            return g_ps

        def emit_proj(hT_tile, lg, t):
            """All 8 chunks, each drained right after its matmuls so psum
            banks free early. Emitted AFTER the DVE tail, so the drain ops
            queue behind the recurrence on every engine."""
            for n in range(NP):
                pj = pj_psp.tile([128, 512], F32, name="pj", tag="pj")
                sl = slice(n * PN, (n + 1) * PN)
                if TAIL_ENG[n] == "A":
                    nc.tensor.matmul(
                        pj[:, :PN],
                        onesB_sb[:],
                        pbrow_sb[:, sl],
                        start=True,
                        stop=False,
                    )
                for k in range(2):
                    nc.tensor.matmul(
                        pj[:, :PN],
                        hT_tile[:, k, :],
                        pw_sb[:, k * VS + n * PN : k * VS + (n + 1) * PN],
                        start=(k == 0) and TAIL_ENG[n] != "A",
                        stop=(k == 1),
                    )
                if TAIL_ENG[n] == "A":
                    nc.scalar.copy(lg[:, sl], pj[:, :PN])
                elif TAIL_ENG[n] == "P":
                    nc.gpsimd.tensor_add(lg[:, sl], pj[:, :PN], pbb_sb[:, sl])
                else:
                    nc.vector.tensor_add(lg[:, sl], pj[:, :PN], pbb_sb[:, sl])
            nc.sync.dma_start(out=logits_out[t], in_=lg[:])

        # ---- prologue: x(0) is the GO row (idx[:,0]=GO_IDX) ----
        xt0 = emit_xt(0)
        g_cur = emit_xpart(xt0)
        hT = h0t_sb
        c_cur = c0t_sb
        prev = None

        MUL = mybir.AluOpType.mult
        ADD = mybir.AluOpType.add
        SUB = mybir.AluOpType.subtract
        AF = mybir.ActivationFunctionType

        for t in range(L):
            # (a) close gates(t): h-passes with hT = h(t-1) [H,B] bf16.
            # j-MAJOR order: all k-half-0 passes first — they only need
            # h_new half 0, which the half-split cell tail (below) produces
            # ~1us before half 1, so these start earlier.
            for j in (0, 1):  # W_hh.T K-chunks (wc slots 0,1)
                for gc in range(NGC):
                    nc.tensor.matmul(
                        g_cur[:, gc, :],
                        wc_sb[:, j * G + gc * 128 : j * G + (gc + 1) * 128],
                        hT[:, j, :],
                        start=False,
                        stop=(j == 1) and (gc % 4 == 3),
                    )

            # (b) xT(t+1) transposes (ready early; SP queue)
            if t + 1 < L:
                xt = emit_xt(t + 1)

            # (d) activations per bank half: sigmoid(i,f), tanh(g),
            # sigmoid(o) — all in the 'sigmoid_and_others' table set, so no
            # table reloads. Chunk order [i0 i1 f0 f1 | g0 g1 o0 o1].
            gact = work.tile([128, NGC, 128], F32, name="gact", tag="gact")
            nc.scalar.activation(gact[:, 0:4, :], g_cur[:, 0:4, :], AF.Sigmoid)
            nc.scalar.activation(gact[:, 4:6, :], g_cur[:, 4:6, :], AF.Tanh)
            nc.scalar.activation(gact[:, 6:8, :], g_cur[:, 6:8, :], AF.Sigmoid)

            # c = f*c + i*tg ; h = o*tanh(c) — plain TensorTensor ops on
            # Pool (the HW compiler rejects TensorScalarPtr there).
            fc = work.tile([128, 2, 128], F32, name="fc", tag="fc")
            itg = work.tile([128, 2, 128], F32, name="itg", tag="itg")
            c_new = state.tile([128, 2, 128], F32, name="c_new", tag="c")
            th = work.tile([128, 2, 128], F32, name="th", tag="th")
            h_new = state.tile([128, 2, 128], BF16, name="h_new", tag="h")
            nc.gpsimd.tensor_mul(fc[:], gact[:, 2:4, :], c_cur[:])
            # itg on DVE: runs concurrently with fc (Pool), shortening the
            # serial cell chain; it precedes the proj drains in the DVE
            # stream but its input (tanh g) is ready early enough that the
            # drains are not materially delayed.
            nc.vector.tensor_mul(itg[:], gact[:, 0:2, :], gact[:, 4:6, :])
            # c/th split per H-half pipelines Pool and ACT; h_new stays ONE
            # op so every next-step h-pass becomes ready simultaneously and
            # the scheduler keeps them bank-major (bank0 closes early -> the
            # sigmoid chain starts sooner). Pool touches SBUF only (PSUM is
            # off-limits to GPSIMD on real HW).
            # Half-split pipelined tail: c/th/h for k=0 complete before k=1
            # starts, so the next step's j=0 h-passes fire ~1us earlier.
            for k in (0, 1):
                nc.gpsimd.tensor_add(c_new[:, k, :], fc[:, k, :], itg[:, k, :])
                nc.scalar.activation(th[:, k, :], c_new[:, k, :], AF.Tanh)
                nc.gpsimd.tensor_mul(
                    h_new[:, k, :], gact[:, 6 + k, :], th[:, k, :]
                )
            c_cur = c_new

            # (e) projection(t-1): mms + interleaved drains + stream-out
            if prev is not None:
                lg_prev = lgp.tile([B, VS], BF16, name="lg", tag="lg")
                emit_proj(prev, lg_prev, t - 1)

            # (f) open gates(t+1) LAST on PE: by now the sigmoids have read
            # gates(t), so one psum pair suffices (bufs=1) freeing banks
            # for the projection pipeline.
            g_next = emit_xpart(xt) if t + 1 < L else None

            # gather LAST: its SWDGE generation (~1us on the Pool engine)
            # sits AFTER the cell ops in the Pool stream. (Scheduler note:
            # emission position and high_priority offsets do NOT move it —
            # the Tile scheduler orders the Pool stream by readiness, so
            # the gather lands ahead of fc and costs ~0.6us/step; batched
            # gathers and wait-hints both regress, see memory notes.)
            if t + LOOKAHEAD < L:
                gather(t + LOOKAHEAD)

            prev = hT = h_new
            g_cur = g_next

        lg_last = lgp.tile([B, VS], BF16, name="lg", tag="lg")
        emit_proj(prev, lg_last, L - 1)


def build_program(reps=1):
    """Build + compile the Bacc program. reps>1 repeats the whole kernel
    body (for slope-based HW timing)."""
    nc = bacc.Bacc("TRN2", target_bir_lowering=False, debug=False,
                   enable_asserts=False)
    io = {
        "idx": nc.dram_tensor("idx", [B, L], I32, kind="ExternalInput")[:],
        "h0t": nc.dram_tensor("h0t", [2, 128, 128], BF16, kind="ExternalInput")[:],
        "c0t": nc.dram_tensor("c0t", [2, 128, 128], F32, kind="ExternalInput")[:],
        "wc": nc.dram_tensor("wc", [4, 128, G], BF16, kind="ExternalInput")[:],
        "bgate": nc.dram_tensor("bgate", [1, G], BF16, kind="ExternalInput")[:],
        "onesb": nc.dram_tensor("onesb", [1, 128], BF16, kind="ExternalInput")[:],
        "pw": nc.dram_tensor("pw", [2, 128, VS], BF16, kind="ExternalInput")[:],
        "pbb": nc.dram_tensor("pbb", [B, VS], F32, kind="ExternalInput")[:],
        "pbrow": nc.dram_tensor("pbrow", [1, VS], BF16, kind="ExternalInput")[:],
        "embed": nc.dram_tensor("embed", [VOCAB, H], BF16, kind="ExternalInput")[:],
        "logits": nc.dram_tensor("logits", [L, B, VS], BF16, kind="ExternalOutput")[:],
    }
    with tile.TileContext(nc) as tc:
        for _ in range(reps):
            emit_kernel(tc, io)
    nc.compile()
    return nc


def make_in_maps(inputs):
    bf = ml_dtypes.bfloat16
    outputs = np.asarray(inputs["outputs"])
    h0 = np.asarray(inputs["h0"], dtype=np.float32)
    c0 = np.asarray(inputs["c0"], dtype=np.float32)
    embed_W = np.asarray(inputs["embed_W"], dtype=np.float32)
    W_ih = np.asarray(inputs["W_ih"], dtype=np.float32)
    W_hh = np.asarray(inputs["W_hh"], dtype=np.float32)
    b = (np.asarray(inputs["b_ih"], dtype=np.float32)
         + np.asarray(inputs["b_hh"], dtype=np.float32))
    proj_W = np.asarray(inputs["proj_W"], dtype=np.float32)
    proj_b = np.asarray(inputs["proj_b"], dtype=np.float32)

    idx = outputs.T.astype(np.int64).copy()  # [B, L]
    idx[:, 0] = GO_IDX
    idx = np.clip(idx, 0, VOCAB - 1).astype(np.int32)

    WhhT = np.ascontiguousarray(W_hh.T)  # [256, 1024]
    WihT = np.ascontiguousarray(W_ih.T)
    wc = np.stack([WhhT[0:128], WhhT[128:256], WihT[0:128], WihT[128:256]])
    bgate = b.copy()
    wc = np.ascontiguousarray(wc).astype(bf)
    bgate = np.ascontiguousarray(bgate[None, :]).astype(bf)

    h0t = np.stack([h0.T[0:128], h0.T[128:256]]).astype(bf)  # [2,128,B]
    c0t = np.ascontiguousarray(np.stack([c0.T[0:128], c0.T[128:256]]))

    onesb = np.ones((1, 128), dtype=np.float32).astype(bf)
    pwT = np.ascontiguousarray(proj_W.T)  # [256, 32000]

    common = dict(idx=idx, h0t=h0t, c0t=c0t, wc=wc, bgate=bgate, onesb=onesb,
                  embed=np.ascontiguousarray(embed_W.astype(bf)))
    in_maps = []
    for c in range(NCORES):
        sl = slice(c * VS, (c + 1) * VS)
        in_maps.append(dict(
            common,
            pw=np.ascontiguousarray(
                np.stack([pwT[0:128, sl], pwT[128:256, sl]])).astype(bf),
            pbb=np.ascontiguousarray(np.tile(proj_b[None, sl], (B, 1))),
            pbrow=np.ascontiguousarray(proj_b[None, sl]).astype(bf),
        ))
    return in_maps


_NC_CACHE = {}


def postprocess(res) -> np.ndarray:
    return np.concatenate(
        [res.results[c]["logits"] for c in range(NCORES)], axis=2
    ).astype(np.float32)


def kernel(**inputs) -> np.ndarray:
    if "nc" not in _NC_CACHE:
        _NC_CACHE["nc"] = build_program()
    nc = _NC_CACHE["nc"]
    in_maps = make_in_maps(inputs)
    res = run_bass_kernel_spmd(nc, in_maps, list(range(NCORES)))
    return postprocess(res)

